# revision 1
# baseline (speedup 1.0000x reference)
"""Fused attention kernel for TRN2, SPMD across 8 NeuronCores.

Problem: out = softmax(mask ? (Q Wq^T + bq)(K Wk^T + bk)^T / sqrt(D) : -1e9)
               @ (V Wv^T + bv)
with B=4, L=2048, E=D=1024.

Sharding: core c handles batch b=c//2, query-half h=c%2 (1024 query rows).
No collectives needed; K/V rows for the batch are fully loaded per core.

Algebra (per core; Xq = Q-shard (1024,E), Xk = K[b] (2048,E), Xv = V[b]):
  scores = (Xq @ Wqk) @ Xk^T + 1 (x) w^T          Wqk = Wq^T Wk / 32
                                                  w   = Xk @ (Wk^T bq) / 32
  (q.bk and bq.bk terms are per-query-row constants and cancel in softmax;
  masked softmax realized as p = exp(s); p *= mask; p /= sum(p) — scores are
  O(1) so no max-subtraction is needed)
  out = (attn @ Xv) @ Wv^T + 1 (x) bv             (rows of attn sum to 1)

float32r (TF32-like, full PE rate at free>=256) for phases 0/1; bf16 for the
scores/AV/out-projection matmuls. All biases folded in as K=1 matmuls.

K_STAGES env (debug): 1=stage A only, 2=+XkT/w, 3=+WvT, 4=+Vb, 5=full.
"""
import os
from contextlib import ExitStack

import numpy as np

import concourse.bacc as bacc
import concourse.tile as tile
from concourse import mybir
from concourse.bass_utils import run_bass_kernel_spmd
from concourse.masks import make_identity

F32 = mybir.dt.float32
F32R = mybir.dt.float32r
BF16 = mybir.dt.bfloat16
I32 = mybir.dt.int32
AF = mybir.ActivationFunctionType
ALU = mybir.AluOpType

B, L, E, D = 4, 2048, 1024, 1024
LS = 1024          # query rows per core
J = 2048           # key rows per core
P = 128
NCORES = 8
SCALE = 1.0 / 32.0  # 1/sqrt(D)

EC = E // P        # 8 chunks of 128 along E/D dims
JC = J // P        # 16 chunks along J
LT = LS // P       # 8 query tiles per core

STAGES = int(os.environ.get("K_STAGES", "5"))


def _transpose_chunks(nc, ps_tr, src, dst_fn, nblk, ident, psdt, lbl,
                      dve_frac=2):
    """Transpose nblk [P,P] blocks of src (groups of 4 share a psum bank).

    src: AP [P, nblk*P]; dst_fn(i) -> destination AP [P, P] for block i.
    dve_frac of every 4 evictions go to DVE, the rest to ACT.
    """
    for t0 in range(0, nblk, 4):
        ps = ps_tr.tile([P, 512], psdt, name=f"pstr_{lbl}", tag="tr")
        for k in range(4):
            nc.tensor.transpose(
                ps[:, k * P:(k + 1) * P],
                src[:, (t0 + k) * P:(t0 + k + 1) * P],
                ident[:],
            )
        for k in range(4):
            dst = dst_fn(t0 + k)
            srcp = ps[:, k * P:(k + 1) * P]
            if (t0 // 4 + k) % 2 == 0:
                nc.vector.tensor_copy(dst, srcp)
            else:
                nc.scalar.activation(out=dst, in_=srcp, func=AF.Copy)


def _build():
    nc = bacc.Bacc(None, target_bir_lowering=False)

    Xq_e = nc.declare_dram_parameter("XqT", [E, LS], BF16, isOutput=False)
    Xk_e = nc.declare_dram_parameter("XkT", [E, J], BF16, isOutput=False)
    Xv_e = nc.declare_dram_parameter("Xv", [J, E], BF16, isOutput=False)
    Mk_e = nc.declare_dram_parameter("mask", [LS, J], BF16, isOutput=False)
    Wqk_e = nc.declare_dram_parameter("Wqk", [E, E], BF16, isOutput=False)
    kb_e = nc.declare_dram_parameter("kb", [E], F32, isOutput=False)
    Wv_e = nc.declare_dram_parameter("WvT", [E, D], BF16, isOutput=False)
    bv_e = nc.declare_dram_parameter("bv", [D], F32, isOutput=False)
    out_e = nc.declare_dram_parameter("out", [LS, D], F32, isOutput=True)

    # chunked DRAM views: [p, chunk, free]
    XqT_d = Xq_e.ap().rearrange("(c p) l -> p c l", p=P)
    XkT_d = Xk_e.ap().rearrange("(c p) j -> p c j", p=P)
    Xv_d = Xv_e.ap().rearrange("(c p) e -> p c e", p=P)
    Wqk_d = Wqk_e.ap().rearrange("(c p) e -> p c e", p=P)
    kb_d = kb_e.ap().rearrange("(c p) -> p c", p=P)
    WvT_d = Wv_e.ap().rearrange("(c p) d -> p c d", p=P)
    Mk_d = Mk_e.ap().rearrange("(c p) j -> p c j", p=P)
    bv_d = bv_e.ap().rearrange("(o d) -> o d", o=1)
    out_d = out_e.ap().rearrange("(c p) d -> p c d", p=P)

    with tile.TileContext(nc) as tc, ExitStack() as long_pools:
        lp_pool = lambda name: long_pools.enter_context(
            tc.tile_pool(name=name, bufs=1))
        with (
            tc.tile_pool(name="ps_s", bufs=2, space="PSUM") as ps_s,
            tc.tile_pool(name="ps_mm", bufs=2, space="PSUM") as ps_mm,
            tc.tile_pool(name="ps_tr", bufs=3, space="PSUM") as ps_tr,
        ):
            # ---- constants ----
            consts = lp_pool("consts")
            ident_f = consts.tile([P, P], F32, name="ident_f")
            make_identity(nc, ident_f[:])
            ident_r = consts.tile([P, P], F32R, name="ident_r")
            nc.scalar.activation(out=ident_r[:], in_=ident_f[:], func=AF.Copy)
            ident_b = consts.tile([P, P], BF16, name="ident_b")
            nc.vector.tensor_copy(ident_b[:], ident_f[:])

            bvb_sb = consts.tile([P, D], F32, name="bvb_sb")
            kb_sb = consts.tile([P, EC], F32, name="kb_sb")

            tT_sb = lp_pool("tT_p").tile([P, EC, LS], BF16, name="tT_sb")

            # PE warmup: no-DMA transposes fill the initial DMA-latency
            # window and bring the PE out of its cold p-state before the
            # first real matmuls
            for wu in range(12):
                ps = ps_tr.tile([P, 512], F32, name="pswu", tag="tr")
                for k in range(4):
                    nc.tensor.transpose(ps[:, k * P:(k + 1) * P],
                                        ident_f[:], ident_f[:])

            # ===== stage A+B: Wqk ; kb ; XqT ; phase 1 ; XkT ; w =====
            if STAGES >= 2:
                XkT_sb = lp_pool("XkT_p").tile([P, EC, J], BF16,
                                               name="XkT_sb")
            with (
                tc.tile_pool(name="wqk_pool", bufs=1) as wqk_pool,
                tc.tile_pool(name="xqt_pool", bufs=1) as xqt_pool,
            ):
                wqk_sb = wqk_pool.tile([P, EC, E], BF16, name="wqk_sb")
                xqT_sb = xqt_pool.tile([P, EC, LS], BF16, name="xqT_sb")
                nc.sync.dma_start(out=kb_sb[:], in_=kb_d)
                for c in range(EC):
                    nc.sync.dma_start(out=wqk_sb[:, c, :],
                                      in_=Wqk_d[:, c, :])
                    nc.scalar.dma_start(out=xqT_sb[:, c, :],
                                        in_=XqT_d[:, c, :])
                import concourse.bass as _bass
                bv_bcast = _bass.AP(tensor=bv_e, offset=0,
                                    ap=[[0, P], [1, D]])
                nc.scalar.dma_start(out=bvb_sb[:], in_=bv_bcast)

                if STAGES >= 2:
                    # ===== phase 1 interleaved with XkT transposes =====
                    def emit_xkt(et):
                        eng = nc.sync if et % 2 == 0 else nc.scalar
                        eng.dma_start(out=XkT_sb[:, et, :],
                                      in_=XkT_d[:, et, :])

                    for e2t in range(EC):
                        # phase 1: tT = (Xq @ Wqk)^T  [e2, l] bf16
                        for lc in range(2):
                            ps = ps_mm.tile([P, 512], F32, name="ps1",
                                            tag="mm")
                            for e1t in range(EC):
                                nc.tensor.matmul(
                                    ps[:],
                                    wqk_sb[:, e1t, e2t * P:(e2t + 1) * P],
                                    xqT_sb[:, e1t, lc * 512:(lc + 1) * 512],
                                    start=(e1t == 0), stop=(e1t == EC - 1),
                                )
                            nc.vector.tensor_scalar(
                                out=tT_sb[:, e2t, lc * 512:(lc + 1) * 512],
                                in0=ps[:],
                                scalar1=kb_sb[:, e2t:e2t + 1],
                                scalar2=None,
                                op0=ALU.add,
                            )
                        emit_xkt(e2t)



            def emit_stage_c():
                # ===== stage C: WvT [d, do] bf16 direct loads =====
                for dt in range(EC):
                    eng = nc.sync if dt % 2 == 0 else nc.scalar
                    eng.dma_start(out=WvT_sb[:, dt, :], in_=WvT_d[:, dt, :])

            def emit_stage_d():
                # ===== stage D: Vb = Xv natural [j, d] (bf16 from host) ====
                for jt in range(JC):
                    eng = nc.sync if jt % 2 == 0 else nc.scalar
                    eng.dma_start(out=Vb_sb[:, jt, :], in_=Xv_d[:, jt, :])

            if STAGES >= 3:
                WvT_sb = lp_pool("WvT_p").tile([P, EC, D], BF16,
                                               name="WvT_sb")
            if STAGES >= 4:
                Vb_sb = lp_pool("Vb_p").tile([P, JC, D], BF16, name="Vb_sb")
            if STAGES >= 3 and STAGES < 5:
                emit_stage_c()
            if STAGES >= 4 and STAGES < 5:
                emit_stage_d()

            if STAGES >= 5:
                # ===== main loop pools =====
                mfp = lp_pool("mf")
                ppool = lp_pool("pp")
                phpool = lp_pool("php")
                ptpool = lp_pool("ptp")
                dnp = lp_pool("dn")

                def emit_mask(lt):
                    maskf = mfp.tile([P, J], BF16, name="maskf", tag="mf",
                                     bufs=2)
                    nc.sync.dma_start(out=maskf[:], in_=Mk_d[:, lt, :])
                    return maskf

                def emit_scores(lt):
                    # phase 2 + exp -> p_sb f32 [P, J]
                    p_sb = ppool.tile([P, J], F32, name="p_sb", tag="p",
                                      bufs=2)
                    for jt4 in range(4):
                        ps = ps_s.tile([P, 512], F32, name="ps_sc", tag="s",
                                       bufs=3)
                        for e2t in range(EC):
                            nc.tensor.matmul(
                                ps[:],
                                tT_sb[:, e2t, lt * P:(lt + 1) * P],
                                XkT_sb[:, e2t, jt4 * 512:(jt4 + 1) * 512],
                                start=(e2t == 0), stop=(e2t == EC - 1),
                            )
                        nc.scalar.activation(
                            out=p_sb[:, jt4 * 512:(jt4 + 1) * 512],
                            in_=ps[:], func=AF.Exp,
                        )
                    return p_sb

                def emit_softmax_pt(lt, lh, p_sb, maskf, pT_sb):
                    # masked sum -> denom; p *= mask; normalize -> bf16; pT
                    denom = dnp.tile([P, 1], F32, name="denom", tag="dn",
                                     bufs=4)
                    nc.vector.scalar_tensor_tensor(
                        out=p_sb[:], in0=p_sb[:], scalar=1.0, in1=maskf[:],
                        op0=ALU.mult, op1=ALU.mult, accum_out=denom[:],
                    )
                    rden = dnp.tile([P, 1], F32, name="rden", tag="rd",
                                    bufs=4)
                    nc.vector.reciprocal(out=rden[:], in_=denom[:])
                    ph_sb = phpool.tile([P, J], BF16, name="ph_sb", tag="ph",
                                        bufs=2)
                    nhalf = 2 if lt == LT - 1 else 1
                    step = J // nhalf
                    for h in range(nhalf):
                        sl = slice(h * step, (h + 1) * step)
                        nc.vector.tensor_scalar_mul(ph_sb[:, sl],
                                                    p_sb[:, sl], rden[:])
                        _transpose_chunks(
                            nc, ps_tr, ph_sb[:, sl],
                            lambda jt, lh=lh, h=h: pT_sb[
                                :, h * (JC // nhalf) + jt,
                                lh * P:(lh + 1) * P],
                            JC // nhalf, ident_b, BF16, "ph",
                        )

                def emit_pair_scores(lpair):
                    pT_sb = ptpool.tile([P, JC, 2 * P], BF16, name="pT_sb",
                                        tag="pt", bufs=2)
                    lts = [2 * lpair, 2 * lpair + 1]
                    maskfs = [emit_mask(lt) for lt in lts]
                    p_sbs = [emit_scores(lt) for lt in lts]
                    return pT_sb, p_sbs, maskfs

                def emit_pair_soft(lpair, st):
                    pT_sb, p_sbs, maskfs = st
                    lts = [2 * lpair, 2 * lpair + 1]
                    for lh in range(2):
                        emit_softmax_pt(lts[lh], lh, p_sbs[lh], maskfs[lh],
                                        pT_sb)
                    return pT_sb

                def emit_pair_back(lpair, pT_sb):
                    # phase 4: zT [d, l-pair] = Xv^T p^T  (bf16)
                    # last pair: split by l-half so ph4(lh0) overlaps
                    # softmax(lh1) and the tail drains sooner
                    zT_sb = ztpool.tile([P, EC, 2 * P], BF16, name="zT_sb",
                                        tag="zt", bufs=2)
                    halves = ([(0, 2 * P)] if lpair < LT // 2 - 1
                              else [(0, P), (P, 2 * P)])
                    for h0, h1 in halves:
                        for dt in range(EC):
                            ps = ps_mm.tile([P, 512], F32, name="ps4",
                                            tag="mm")
                            for jt in range(JC):
                                nc.tensor.matmul(
                                    ps[:, 0:h1 - h0],
                                    Vb_sb[:, jt, dt * P:(dt + 1) * P],
                                    pT_sb[:, jt, h0:h1],
                                    start=(jt == 0), stop=(jt == JC - 1),
                                )
                            nc.scalar.activation(out=zT_sb[:, dt, h0:h1],
                                                 in_=ps[:, 0:h1 - h0],
                                                 func=AF.Copy)

                    # phase 5: out = zT^T WvT + bv
                    for lh in range(2):
                        lt = 2 * lpair + lh
                        o_sb = opool.tile([P, D], F32, name="o_sb", tag="o",
                                          bufs=3)
                        for doc in range(2):
                            ps = ps_mm.tile([P, 512], F32, name="ps5",
                                            tag="mm")
                            for dt in range(EC):
                                nc.tensor.matmul(
                                    ps[:],
                                    zT_sb[:, dt, lh * P:(lh + 1) * P],
                                    WvT_sb[:, dt, doc * 512:(doc + 1) * 512],
                                    start=(dt == 0), stop=(dt == EC - 1),
                                )
                            nc.vector.tensor_add(
                                o_sb[:, doc * 512:(doc + 1) * 512],
                                ps[:],
                                bvb_sb[:, doc * 512:(doc + 1) * 512],
                            )
                        eng = nc.sync if lt % 2 == 0 else nc.scalar
                        eng.dma_start(out=out_d[:, lt, :], in_=o_sb[:])

                # software pipeline across pairs: scores(k+1) emitted
                # before softmax/pT(k) so PE never starves on the softmax
                # chain; stages C/D overlap pair-0 scores
                pT = emit_pair_soft(0, emit_pair_scores(0))
                emit_stage_d()
                emit_stage_c()
                ztpool = lp_pool("ztp")
                opool = lp_pool("op")
                emit_pair_back(0, pT)
                for lpair in range(1, LT // 2):
                    pT = emit_pair_soft(lpair, emit_pair_scores(lpair))
                    emit_pair_back(lpair, pT)

            if STAGES < 5:
                # debug: write junk so `out` is produced
                with tc.tile_pool(name="dbg", bufs=1) as dbg:
                    o_sb = dbg.tile([P, D], F32, name="o_dbg")
                    nc.vector.memset(o_sb[:], 0.0)
                    nc.vector.tensor_copy(o_sb[:, 0:EC],
                                          tT_sb[:, 0, 0:EC])
                    for lt in range(LT):
                        eng = nc.sync if lt % 2 == 0 else nc.scalar
                        eng.dma_start(out=out_d[:, lt, :], in_=o_sb[:])

    nc.compile()
    return nc


_NC_CACHE = {}


def _get_nc():
    if "nc" not in _NC_CACHE:
        _NC_CACHE["nc"] = _build()
    return _NC_CACHE["nc"]


def _shard_inputs(Q, K, V, mask, Wq_w, Wq_b, Wk_w, Wk_b, Wv_w, Wv_b):
    import ml_dtypes
    bf16 = ml_dtypes.bfloat16
    f32 = np.float32
    Wq32 = np.asarray(Wq_w, f32)
    Wk32 = np.asarray(Wk_w, f32)
    common = {
        "Wqk": np.ascontiguousarray(
            ((Wq32.T @ Wk32) / 32.0).astype(bf16)),
        "kb": np.ascontiguousarray(
            (Wk32.T @ np.asarray(Wq_b, f32)) / 32.0, f32),
        "WvT": np.ascontiguousarray(np.asarray(Wv_w, f32).astype(bf16).T),
        "bv": np.ascontiguousarray(Wv_b, f32),
    }
    in_maps = []
    for c in range(NCORES):
        b, h = divmod(c, 2)
        sl = slice(h * LS, (h + 1) * LS)
        in_maps.append({
            "XqT": np.ascontiguousarray(
                np.asarray(Q[b, sl, :], f32).astype(bf16).T),
            "XkT": np.ascontiguousarray(
                np.asarray(K[b], f32).astype(bf16).T),
            "Xv": np.ascontiguousarray(np.asarray(V[b], f32).astype(bf16)),
            "mask": np.ascontiguousarray(
                np.asarray(mask[b, sl, :]).astype(bf16)),
            **common,
        })
    return in_maps


def _run(inputs, trace=False):
    nc = _get_nc()
    in_maps = _shard_inputs(**inputs)
    res = run_bass_kernel_spmd(nc, in_maps, core_ids=list(range(NCORES)),
                               trace=trace)
    out = np.empty((B, L, D), np.float32)
    for c in range(NCORES):
        b, h = divmod(c, 2)
        out[b, h * LS:(h + 1) * LS, :] = res.results[c]["out"]
    return out, res


def kernel(**inputs):
    out, _ = _run(inputs, trace=False)
    return out



# revision 7
# speedup vs baseline: 1.1828x; 1.1828x over previous
"""Fused attention kernel for TRN2, SPMD across 8 NeuronCores.

Problem: out = softmax(mask ? (Q Wq^T + bq)(K Wk^T + bk)^T / sqrt(D) : -1e9)
               @ (V Wv^T + bv)
with B=4, L=2048, E=D=1024.

Sharding: core c handles batch b=c//2, query-half h=c%2 (1024 query rows).
No collectives needed; K/V rows for the batch are fully loaded per core.

Algebra (per core; Xq = Q-shard (1024,E), Xk = K[b] (2048,E), Xv = V[b]):
  scores = (Xq @ Wqk) @ Xk^T + 1 (x) w^T          Wqk = Wq^T Wk / 32
                                                  w   = Xk @ (Wk^T bq) / 32
  (q.bk and bq.bk terms are per-query-row constants and cancel in softmax;
  masked softmax realized as p = exp(s); p *= mask; p /= sum(p) — scores are
  O(1) so no max-subtraction is needed)
  out = (attn @ Xv) @ Wv^T + 1 (x) bv             (rows of attn sum to 1)

float32r (TF32-like, full PE rate at free>=256) for phases 0/1; bf16 for the
scores/AV/out-projection matmuls. All biases folded in as K=1 matmuls.

K_STAGES env (debug): 1=stage A only, 2=+XkT/w, 3=+WvT, 4=+Vb, 5=full.
"""
import os
from contextlib import ExitStack

import numpy as np

import concourse.bacc as bacc
import concourse.tile as tile
from concourse import mybir
from concourse.bass_utils import run_bass_kernel_spmd
from concourse.masks import make_identity

F32 = mybir.dt.float32
F32R = mybir.dt.float32r
BF16 = mybir.dt.bfloat16
FP8 = mybir.dt.float8e4
I32 = mybir.dt.int32
AF = mybir.ActivationFunctionType
ALU = mybir.AluOpType
DR = mybir.MatmulPerfMode.DoubleRow

B, L, E, D = 4, 2048, 1024, 1024
LS = 1024          # query rows per core
J = 2048           # key rows per core
P = 128
NCORES = 8
SCALE = 1.0 / 32.0  # 1/sqrt(D)

EC = E // P        # 8 chunks of 128 along E/D dims
JC = J // P        # 16 chunks along J
LT = LS // P       # 8 query tiles per core

STAGES = int(os.environ.get("K_STAGES", "5"))


def _transpose_chunks(nc, ps_tr, src, dst_fn, nblk, ident, psdt, lbl,
                      dve_frac=2):
    """Transpose nblk [P,P] blocks of src (groups of 4 share a psum bank).

    src: AP [P, nblk*P]; dst_fn(i) -> destination AP [P, P] for block i.
    dve_frac of every 4 evictions go to DVE, the rest to ACT.
    """
    for t0 in range(0, nblk, 4):
        ps = ps_tr.tile([P, 512], psdt, name=f"pstr_{lbl}", tag="tr")
        for k in range(4):
            nc.tensor.transpose(
                ps[:, k * P:(k + 1) * P],
                src[:, (t0 + k) * P:(t0 + k + 1) * P],
                ident[:],
            )
        for k in range(4):
            dst = dst_fn(t0 + k)
            srcp = ps[:, k * P:(k + 1) * P]
            if (t0 // 4 + k) % 2 == 0:
                nc.vector.tensor_copy(dst, srcp)
            else:
                nc.scalar.activation(out=dst, in_=srcp, func=AF.Copy)


def _build():
    nc = bacc.Bacc(None, target_bir_lowering=False)

    Xq_e = nc.declare_dram_parameter("XqT", [E, LS], BF16, isOutput=False)
    Xk_e = nc.declare_dram_parameter("XkT", [E, J], FP8, isOutput=False)
    Xv_e = nc.declare_dram_parameter("Xv", [J, E], BF16, isOutput=False)
    Mk_e = nc.declare_dram_parameter("mask", [LS, J], BF16, isOutput=False)
    Wqk_e = nc.declare_dram_parameter("Wqk", [E, E], BF16, isOutput=False)
    kb_e = nc.declare_dram_parameter("kb", [E], F32, isOutput=False)
    Wv_e = nc.declare_dram_parameter("WvT", [E, D], BF16, isOutput=False)
    bv_e = nc.declare_dram_parameter("bv", [D], F32, isOutput=False)
    out_e = nc.declare_dram_parameter("out", [LS, D], F32, isOutput=True)

    # chunked DRAM views: [p, chunk, free]
    XqT_d = Xq_e.ap().rearrange("(c p) l -> p c l", p=P)
    XkT_d = Xk_e.ap().rearrange("(c p) j -> p c j", p=P)
    Xv_d = Xv_e.ap().rearrange("(c p) e -> p c e", p=P)
    Wqk_d = Wqk_e.ap().rearrange("(c p) e -> p c e", p=P)
    kb_d = kb_e.ap().rearrange("(c p) -> p c", p=P)
    WvT_d = Wv_e.ap().rearrange("(c p) d -> p c d", p=P)
    Mk_d = Mk_e.ap().rearrange("(c p) j -> p c j", p=P)
    bv_d = bv_e.ap().rearrange("(o d) -> o d", o=1)
    out_d = out_e.ap().rearrange("(c p) d -> p c d", p=P)

    with tile.TileContext(nc) as tc, ExitStack() as long_pools:
        lp_pool = lambda name: long_pools.enter_context(
            tc.tile_pool(name=name, bufs=1))
        with (
            tc.tile_pool(name="ps_s", bufs=2, space="PSUM") as ps_s,
            tc.tile_pool(name="ps_mm", bufs=2, space="PSUM") as ps_mm,
            tc.tile_pool(name="ps_tr", bufs=3, space="PSUM") as ps_tr,
        ):
            # ---- constants ----
            consts = lp_pool("consts")
            ident_f = consts.tile([P, P], F32, name="ident_f")
            make_identity(nc, ident_f[:])
            ident_r = consts.tile([P, P], F32R, name="ident_r")
            nc.scalar.activation(out=ident_r[:], in_=ident_f[:], func=AF.Copy)
            ident_b = consts.tile([P, P], BF16, name="ident_b")
            nc.vector.tensor_copy(ident_b[:], ident_f[:])

            bvb_sb = consts.tile([P, D], F32, name="bvb_sb")
            kb_sb = consts.tile([P, EC], F32, name="kb_sb")

            tT_sb = lp_pool("tT_p").tile([P, EC, LS], FP8, name="tT_sb")

            # PE warmup: no-DMA transposes fill the initial DMA-latency
            # window and bring the PE out of its cold p-state before the
            # first real matmuls
            for wu in range(12):
                ps = ps_tr.tile([P, 512], F32, name="pswu", tag="tr")
                for k in range(4):
                    nc.tensor.transpose(ps[:, k * P:(k + 1) * P],
                                        ident_f[:], ident_f[:])

            # ===== stage A+B: Wqk ; kb ; XqT ; phase 1 ; XkT ; w =====
            if STAGES >= 2:
                XkT_sb = lp_pool("XkT_p").tile([P, EC, J], FP8,
                                               name="XkT_sb")
            with (
                tc.tile_pool(name="wqk_pool", bufs=1) as wqk_pool,
                tc.tile_pool(name="xqt_pool", bufs=1) as xqt_pool,
            ):
                wqk_sb = wqk_pool.tile([P, EC, E], BF16, name="wqk_sb")
                xqT_sb = xqt_pool.tile([P, EC, LS], BF16, name="xqT_sb")
                nc.sync.dma_start(out=kb_sb[:], in_=kb_d)
                for c in range(EC):
                    nc.sync.dma_start(out=wqk_sb[:, c, :],
                                      in_=Wqk_d[:, c, :])
                    nc.scalar.dma_start(out=xqT_sb[:, c, :],
                                        in_=XqT_d[:, c, :])
                import concourse.bass as _bass
                bv_bcast = _bass.AP(tensor=bv_e, offset=0,
                                    ap=[[0, P], [1, D]])
                nc.scalar.dma_start(out=bvb_sb[:], in_=bv_bcast)

                if STAGES >= 2:
                    # ===== phase 1 interleaved with XkT transposes =====
                    def emit_xkt(et):
                        eng = nc.sync if et % 2 == 0 else nc.scalar
                        eng.dma_start(out=XkT_sb[:, et, :],
                                      in_=XkT_d[:, et, :])

                    for e2t in range(EC):
                        # phase 1: tT = (Xq @ Wqk)^T  [e2, l] bf16
                        for lc in range(2):
                            ps = ps_mm.tile([P, 512], F32, name="ps1",
                                            tag="mm")
                            for e1t in range(EC):
                                nc.tensor.matmul(
                                    ps[:],
                                    wqk_sb[:, e1t, e2t * P:(e2t + 1) * P],
                                    xqT_sb[:, e1t, lc * 512:(lc + 1) * 512],
                                    start=(e1t == 0), stop=(e1t == EC - 1),
                                )
                            nc.vector.tensor_scalar(
                                out=tT_sb[:, e2t, lc * 512:(lc + 1) * 512],
                                in0=ps[:],
                                scalar1=kb_sb[:, e2t:e2t + 1],
                                scalar2=None,
                                op0=ALU.add,
                            )
                        emit_xkt(e2t)



            def emit_stage_c():
                # ===== stage C: WvT [d, do] bf16 direct loads =====
                for dt in range(EC):
                    eng = nc.sync if dt % 2 == 0 else nc.scalar
                    eng.dma_start(out=WvT_sb[:, dt, :], in_=WvT_d[:, dt, :])

            def emit_stage_d():
                # ===== stage D: Vb = Xv natural [j, d] (bf16 from host) ====
                for jt in range(JC):
                    eng = nc.sync if jt % 2 == 0 else nc.scalar
                    eng.dma_start(out=Vb_sb[:, jt, :], in_=Xv_d[:, jt, :])

            if STAGES >= 3:
                WvT_sb = lp_pool("WvT_p").tile([P, EC, D], BF16,
                                               name="WvT_sb")
            if STAGES >= 4:
                Vb_sb = lp_pool("Vb_p").tile([P, JC, D], BF16, name="Vb_sb")
            if STAGES >= 3 and STAGES < 5:
                emit_stage_c()
            if STAGES >= 4 and STAGES < 5:
                emit_stage_d()

            if STAGES >= 5:
                # ===== main loop pools =====
                mfp = lp_pool("mf")
                ppool = lp_pool("pp")
                phpool = lp_pool("php")
                ptpool = lp_pool("ptp")
                dnp = lp_pool("dn")

                def emit_mask(lt):
                    maskf = mfp.tile([P, J], BF16, name="maskf", tag="mf",
                                     bufs=2)
                    nc.sync.dma_start(out=maskf[:], in_=Mk_d[:, lt, :])
                    return maskf

                def emit_scores(lt):
                    # phase 2 + exp -> p_sb f32 [P, J]
                    p_sb = ppool.tile([P, J], F32, name="p_sb", tag="p",
                                      bufs=2)
                    for jt4 in range(4):
                        ps = ps_s.tile([P, 512], F32, name="ps_sc", tag="s",
                                       bufs=3)
                        for e2p in range(EC // 2):
                            nc.tensor.matmul(
                                ps[:],
                                tT_sb[:, 2 * e2p:2 * e2p + 2,
                                      lt * P:(lt + 1) * P],
                                XkT_sb[:, 2 * e2p:2 * e2p + 2,
                                       jt4 * 512:(jt4 + 1) * 512],
                                start=(e2p == 0), stop=(e2p == EC // 2 - 1),
                                perf_mode=DR,
                            )
                        nc.scalar.activation(
                            out=p_sb[:, jt4 * 512:(jt4 + 1) * 512],
                            in_=ps[:], func=AF.Exp, scale=SCALE,
                        )
                    return p_sb

                def emit_softmax_pt(lt, lh, p_sb, maskf, pT_sb):
                    # masked sum -> denom; p *= mask; normalize -> bf16; pT
                    denom = dnp.tile([P, 1], F32, name="denom", tag="dn",
                                     bufs=4)
                    nc.vector.scalar_tensor_tensor(
                        out=p_sb[:], in0=p_sb[:], scalar=1.0, in1=maskf[:],
                        op0=ALU.mult, op1=ALU.mult, accum_out=denom[:],
                    )
                    rden = dnp.tile([P, 1], F32, name="rden", tag="rd",
                                    bufs=4)
                    nc.vector.reciprocal(out=rden[:], in_=denom[:])
                    ph_sb = phpool.tile([P, J], BF16, name="ph_sb", tag="ph",
                                        bufs=2)
                    nhalf = 2 if lt == LT - 1 else 1
                    step = J // nhalf
                    for h in range(nhalf):
                        sl = slice(h * step, (h + 1) * step)
                        nc.vector.tensor_scalar_mul(ph_sb[:, sl],
                                                    p_sb[:, sl], rden[:])
                        _transpose_chunks(
                            nc, ps_tr, ph_sb[:, sl],
                            lambda jt, lh=lh, h=h: pT_sb[
                                :, h * (JC // nhalf) + jt,
                                lh * P:(lh + 1) * P],
                            JC // nhalf, ident_b, BF16, "ph",
                        )

                def emit_pair_scores(lpair):
                    pT_sb = ptpool.tile([P, JC, 2 * P], BF16, name="pT_sb",
                                        tag="pt", bufs=2)
                    lts = [2 * lpair, 2 * lpair + 1]
                    maskfs = [emit_mask(lt) for lt in lts]
                    p_sbs = [emit_scores(lt) for lt in lts]
                    return pT_sb, p_sbs, maskfs

                def emit_pair_soft(lpair, st):
                    pT_sb, p_sbs, maskfs = st
                    lts = [2 * lpair, 2 * lpair + 1]
                    for lh in range(2):
                        emit_softmax_pt(lts[lh], lh, p_sbs[lh], maskfs[lh],
                                        pT_sb)
                    return pT_sb

                def emit_pair_back(lpair, pT_sb):
                    # phase 4: zT [d, l-pair] = Xv^T p^T  (bf16)
                    # last pair: split by l-half so ph4(lh0) overlaps
                    # softmax(lh1) and the tail drains sooner
                    zT_sb = ztpool.tile([P, EC, 2 * P], BF16, name="zT_sb",
                                        tag="zt", bufs=2)
                    halves = ([(0, 2 * P)] if lpair < LT // 2 - 1
                              else [(0, P), (P, 2 * P)])
                    for h0, h1 in halves:
                        for dt in range(EC):
                            ps = ps_mm.tile([P, 512], F32, name="ps4",
                                            tag="mm")
                            for jt in range(JC):
                                nc.tensor.matmul(
                                    ps[:, 0:h1 - h0],
                                    Vb_sb[:, jt, dt * P:(dt + 1) * P],
                                    pT_sb[:, jt, h0:h1],
                                    start=(jt == 0), stop=(jt == JC - 1),
                                )
                            nc.scalar.activation(out=zT_sb[:, dt, h0:h1],
                                                 in_=ps[:, 0:h1 - h0],
                                                 func=AF.Copy)

                    # phase 5: out = zT^T WvT + bv
                    for lh in range(2):
                        lt = 2 * lpair + lh
                        o_sb = opool.tile([P, D], F32, name="o_sb", tag="o",
                                          bufs=3)
                        for doc in range(2):
                            ps = ps_mm.tile([P, 512], F32, name="ps5",
                                            tag="mm")
                            for dt in range(EC):
                                nc.tensor.matmul(
                                    ps[:],
                                    zT_sb[:, dt, lh * P:(lh + 1) * P],
                                    WvT_sb[:, dt, doc * 512:(doc + 1) * 512],
                                    start=(dt == 0), stop=(dt == EC - 1),
                                )
                            nc.vector.tensor_add(
                                o_sb[:, doc * 512:(doc + 1) * 512],
                                ps[:],
                                bvb_sb[:, doc * 512:(doc + 1) * 512],
                            )
                        eng = nc.sync if lt % 2 == 0 else nc.scalar
                        eng.dma_start(out=out_d[:, lt, :], in_=o_sb[:])

                # software pipeline across pairs: scores(k+1) emitted
                # before softmax/pT(k) so PE never starves on the softmax
                # chain; stages C/D overlap pair-0 scores
                pT = emit_pair_soft(0, emit_pair_scores(0))
                emit_stage_d()
                emit_stage_c()
                ztpool = lp_pool("ztp")
                opool = lp_pool("op")
                emit_pair_back(0, pT)
                for lpair in range(1, LT // 2):
                    pT = emit_pair_soft(lpair, emit_pair_scores(lpair))
                    emit_pair_back(lpair, pT)

            if STAGES < 5:
                # debug: write junk so `out` is produced
                with tc.tile_pool(name="dbg", bufs=1) as dbg:
                    o_sb = dbg.tile([P, D], F32, name="o_dbg")
                    nc.vector.memset(o_sb[:], 0.0)
                    nc.vector.tensor_copy(o_sb[:, 0:EC],
                                          tT_sb[:, 0, 0:EC])
                    for lt in range(LT):
                        eng = nc.sync if lt % 2 == 0 else nc.scalar
                        eng.dma_start(out=out_d[:, lt, :], in_=o_sb[:])

    nc.compile()
    return nc


_NC_CACHE = {}


def _get_nc():
    if "nc" not in _NC_CACHE:
        _NC_CACHE["nc"] = _build()
    return _NC_CACHE["nc"]


def _shard_inputs(Q, K, V, mask, Wq_w, Wq_b, Wk_w, Wk_b, Wv_w, Wv_b):
    import ml_dtypes
    bf16 = ml_dtypes.bfloat16
    fp8 = ml_dtypes.float8_e4m3
    f32 = np.float32
    Wq32 = np.asarray(Wq_w, f32)
    Wk32 = np.asarray(Wk_w, f32)
    # NOTE: the 1/sqrt(D) score scale is applied at the Exp activation
    # (scale=1/32), so Wqk/kb are unscaled here — keeps tT in fp8's
    # normal range (sigma ~ 0.33).
    common = {
        "Wqk": np.ascontiguousarray(
            (Wq32.T @ Wk32).astype(bf16)),
        "kb": np.ascontiguousarray(
            Wk32.T @ np.asarray(Wq_b, f32), f32),
        "WvT": np.ascontiguousarray(np.asarray(Wv_w, f32).astype(bf16).T),
        "bv": np.ascontiguousarray(Wv_b, f32),
    }
    in_maps = []
    for c in range(NCORES):
        b, h = divmod(c, 2)
        sl = slice(h * LS, (h + 1) * LS)
        in_maps.append({
            "XqT": np.ascontiguousarray(
                np.asarray(Q[b, sl, :], f32).astype(bf16).T),
            "XkT": np.ascontiguousarray(
                np.asarray(K[b], f32).astype(fp8).T),
            "Xv": np.ascontiguousarray(np.asarray(V[b], f32).astype(bf16)),
            "mask": np.ascontiguousarray(
                np.asarray(mask[b, sl, :]).astype(bf16)),
            **common,
        })
    return in_maps


def _run(inputs, trace=False):
    nc = _get_nc()
    in_maps = _shard_inputs(**inputs)
    res = run_bass_kernel_spmd(nc, in_maps, core_ids=list(range(NCORES)),
                               trace=trace)
    out = np.empty((B, L, D), np.float32)
    for c in range(NCORES):
        b, h = divmod(c, 2)
        out[b, h * LS:(h + 1) * LS, :] = res.results[c]["out"]
    return out, res


def kernel(**inputs):
    out, _ = _run(inputs, trace=False)
    return out

